# revision 14
# baseline (speedup 1.0000x reference)
"""Causal-intervention attention on 8 trn2 cores.

Sharding: head-parallel. Core c computes heads {2c, 2c+1} for BOTH batches
(so the SPMD program is identical on every core even though the
mask-dependent program structure differs per batch). Each core emits a
partial output y_c = ctx_c @ Wo[rows_c]; the host sums the 8 partials and
adds the (folded) bias.

Mask handling: tokens are sorted by cause_mask on the host (per batch).
scores * (1 - 0.5*s*cm[q]*em[k]) is then exact by using an em-scaled copy
of K^T (K2) for the cm=1 query block and plain K^T for the cm=0 block.

Softmax: exp without max subtraction (scores ~ N(0,1), max |s| < ~7, safe
in fp32). Denominator comes from a ones column appended to V (M=65 AV
matmul); normalization is applied to the attention output before the
output projection.

All matmul operands are bf16 (weights, x^T, Q^T/K^T, exp(S), V, O^T);
accumulation is fp32 in PSUM. fp32r streams at the same rate but its
self-loading LDWEIGHTS only accepts one semaphore wait, which Tile's
schedules exceed.
"""

import numpy as np
import ml_dtypes
from contextlib import ExitStack

import concourse.bass as bass
from concourse import bacc
import concourse.mybir as mybir
import concourse.tile as tile
from concourse.bass_utils import run_bass_kernel_spmd

B, S, D, H = 2, 2048, 1024, 16
HD = D // H  # 64
NCORES = 8
HPC = H // NCORES  # heads per core = 2
DPC = HPC * HD  # head-dim cols per core = 128
P = 128

F32 = mybir.dt.float32
BF16 = mybir.dt.bfloat16

NKT = S // P          # 16 key tiles of 128
NKT2 = NKT // 2       # 8 key-tile pairs
VSTRIDE = 2 * (HD + 1)  # V' storage stride per k-tile: [V_h0|1|V_h1|1] = 130

LAST_RESULTS = None  # BassKernelResults of the most recent run (for test.py)
TRACE = False


def _qslices(n_c1):
    """Query slices: breakpoints at multiples of 512 plus the cm=1/cm=0
    boundary. Each slice is (start, width, use_k2)."""
    pts = sorted(set([0, 512, 1024, 1536, 2048, int(n_c1)]))
    out = []
    for a, b in zip(pts[:-1], pts[1:]):
        out.append((a, b - a, b <= n_c1))
    return out


def _build_program(n_c1):
    """n_c1: tuple of per-batch cm=1 counts (compile-time structure)."""
    nc = bacc.Bacc(None, target_bir_lowering=False)

    xtb = nc.declare_dram_parameter("xtb", [B, D, S], BF16, isOutput=False)
    wq = nc.declare_dram_parameter("wq", [D, DPC], BF16, isOutput=False)
    wk = nc.declare_dram_parameter("wk", [D, DPC], BF16, isOutput=False)
    wvb = nc.declare_dram_parameter("wvb", [D, DPC], BF16, isOutput=False)
    wo = nc.declare_dram_parameter("wo", [DPC, D], BF16, isOutput=False)
    bq = nc.declare_dram_parameter("bq", [DPC, 1], F32, isOutput=False)
    bk = nc.declare_dram_parameter("bk", [DPC, 1], F32, isOutput=False)
    emf = nc.declare_dram_parameter("emf", [B, P, S], F32, isOutput=False)
    y = nc.declare_dram_parameter("y", [B, S, D], F32, isOutput=True)

    with tile.TileContext(nc) as tc, ExitStack() as ctx:
        # ---- pools ----
        xtb_pool = ctx.enter_context(tc.tile_pool(name="xtb", bufs=10))
        wpool = ctx.enter_context(tc.tile_pool(name="w", bufs=1))
        emf_pool = ctx.enter_context(tc.tile_pool(name="emf", bufs=2))
        qk_pool = ctx.enter_context(tc.tile_pool(name="qk", bufs=2))
        v_pool = ctx.enter_context(tc.tile_pool(name="v", bufs=2))
        et_pool = ctx.enter_context(tc.tile_pool(name="et", bufs=4))
        small = ctx.enter_context(tc.tile_pool(name="small", bufs=3))
        y_pool = ctx.enter_context(tc.tile_pool(name="y", bufs=3))
        dram_pool = ctx.enter_context(tc.tile_pool(name="drp", bufs=2, space="DRAM"))
        ps_proj = ctx.enter_context(tc.tile_pool(name="psp", bufs=2, space="PSUM"))
        ps_s = ctx.enter_context(tc.tile_pool(name="pss", bufs=1, space="PSUM"))
        ps_o = ctx.enter_context(tc.tile_pool(name="pso", bufs=1, space="PSUM"))

        # ---- weights / biases (once) ----
        wqs, wks, wvbs = [], [], []
        for kt in range(D // P):
            tq = wpool.tile([P, DPC], BF16, tag=f"wq{kt}", name=f"wq_sb{kt}")
            nc.sync.dma_start(out=tq[:], in_=wq[kt * P:(kt + 1) * P, :])
            wqs.append(tq)
            tk = wpool.tile([P, DPC], BF16, tag=f"wk{kt}", name=f"wk_sb{kt}")
            nc.sync.dma_start(out=tk[:], in_=wk[kt * P:(kt + 1) * P, :])
            wks.append(tk)
            tv = wpool.tile([P, DPC], BF16, tag=f"wv{kt}", name=f"wv_sb{kt}")
            nc.sync.dma_start(out=tv[:], in_=wvb[kt * P:(kt + 1) * P, :])
            wvbs.append(tv)
        wo_sb = wpool.tile([P, D], BF16, tag="wo_sb")
        bq_sb = wpool.tile([P, 1], F32, tag="bq_sb")
        bk_sb = wpool.tile([P, 1], F32, tag="bk_sb")
        nc.sync.dma_start(out=wo_sb[:], in_=wo[:])
        nc.sync.dma_start(out=bq_sb[:], in_=bq[:])
        nc.sync.dma_start(out=bk_sb[:], in_=bk[:])

        for b in range(B):
            # ---- load x^T tiles (bf16) ----
            xtbs = []
            for kt in range(D // P):
                tb = xtb_pool.tile([P, S], BF16, tag="xtb")
                nc.sync.dma_start(out=tb[:], in_=xtb[b, kt * P:(kt + 1) * P, :])
                xtbs.append(tb)
            emf_sb = emf_pool.tile([P, S], F32, tag="emf_sb")
            nc.sync.dma_start(out=emf_sb[:], in_=emf[b])

            # ---- Q^T / K^T projections (d-major [128, S], bf16) ----
            qt_sb = qk_pool.tile([P, S], BF16, tag="qt_sb")
            kt_sb = qk_pool.tile([P, S], BF16, tag="kt_sb")
            k2t_sb = qk_pool.tile([P, S], BF16, tag="k2t_sb")
            ont_sb = qk_pool.tile([P, S], BF16, tag="ont_sb")
            for ns in range(4):
                cs = slice(ns * 512, (ns + 1) * 512)
                psq = ps_proj.tile([P, 512], F32, tag="pp", name=f"psq_{b}_{ns}")
                for kt in range(D // P):
                    nc.tensor.matmul(
                        psq[:], lhsT=wqs[kt][:], rhs=xtbs[kt][:, cs],
                        start=(kt == 0), stop=(kt == D // P - 1))
                nc.vector.tensor_scalar_add(qt_sb[:, cs], psq[:], bq_sb[:])
                psk = ps_proj.tile([P, 512], F32, tag="pp", name=f"psk_{b}_{ns}")
                for kt in range(D // P):
                    nc.tensor.matmul(
                        psk[:], lhsT=wks[kt][:], rhs=xtbs[kt][:, cs],
                        start=(kt == 0), stop=(kt == D // P - 1))
                nc.vector.tensor_scalar_add(kt_sb[:, cs], psk[:], bk_sb[:])
                # K2 = (K + bk) * emfac  (em-scaled copy)
                nc.vector.scalar_tensor_tensor(
                    out=k2t_sb[:, cs], in0=psk[:], scalar=bk_sb[:],
                    in1=emf_sb[:, cs],
                    op0=mybir.AluOpType.add, op1=mybir.AluOpType.mult)

            # ---- V projection (s-major, with ones cols): V'[128, 16*130] ----
            v_sb = v_pool.tile([P, NKT * VSTRIDE], BF16, tag="v_sb")
            nc.vector.memset(v_sb[:], 1.0)
            for st in range(NKT):
                psv = ps_proj.tile([P, 512], F32, tag="pp", name=f"psv_{b}_{st}")
                for kt in range(D // P):
                    nc.tensor.matmul(
                        psv[:, 0:DPC],
                        lhsT=xtbs[kt][:, st * P:(st + 1) * P],
                        rhs=wvbs[kt][:],
                        start=(kt == 0), stop=(kt == D // P - 1))
                o = st * VSTRIDE
                nc.vector.tensor_copy(v_sb[:, o:o + HD], psv[:, 0:HD])
                nc.vector.tensor_copy(v_sb[:, o + HD + 1:o + 2 * HD + 1],
                                      psv[:, HD:2 * HD])

            # ---- attention (+ interleaved output projection) ----
            qsl = _qslices(n_c1[b])
            units = [(j, kt2) for j in range(len(qsl)) for kt2 in range(NKT2)]
            o_ps = {}   # per live qslice: (o0, o1) psum tiles
            pend = {}   # unit -> (e0, e1)
            state = {"st_ready": 0, "st_emitted": 0}

            def emit_scores(u):
                j, kt2 = u
                q0, w, use_k2 = qsl[j]
                src = k2t_sb if use_k2 else kt_sb
                sp0 = ps_s.tile([P, 1024], F32, tag="sp0", name=f"sp0_{b}_{j}_{kt2}")
                sp1 = ps_s.tile([P, 1024], F32, tag="sp1", name=f"sp1_{b}_{j}_{kt2}")
                for i in range(2):
                    kt = 2 * kt2 + i
                    ks = slice(kt * P, (kt + 1) * P)
                    nc.tensor.matmul(
                        sp0[:, i * 512:i * 512 + w],
                        lhsT=src[0:HD, ks], rhs=qt_sb[0:HD, q0:q0 + w],
                        start=True, stop=True, tile_position=(0, 0))
                    nc.tensor.matmul(
                        sp1[:, i * 512:i * 512 + w],
                        lhsT=src[HD:P, ks], rhs=qt_sb[HD:P, q0:q0 + w],
                        start=True, stop=True, tile_position=(64, 0))
                e0 = et_pool.tile([P, 1024], BF16, tag="e0", name=f"e0_{b}_{j}_{kt2}")
                e1 = et_pool.tile([P, 1024], BF16, tag="e1", name=f"e1_{b}_{j}_{kt2}")
                for i in range(2):
                    cs = slice(i * 512, i * 512 + w)
                    nc.scalar.activation(e0[:, cs], sp0[:, cs],
                                         mybir.ActivationFunctionType.Exp)
                    nc.scalar.activation(e1[:, cs], sp1[:, cs],
                                         mybir.ActivationFunctionType.Exp)
                pend[u] = (e0, e1)

            def emit_av(u):
                j, kt2 = u
                q0, w, _ = qsl[j]
                e0, e1 = pend.pop(u)
                if j not in o_ps:
                    o0_t = ps_o.tile([HD + 1, 512], F32, tag="o0", name=f"o0_{b}_{j}")
                    o1_t = ps_o.tile([HD + 1, 512], F32, tag="o1", name=f"o1_{b}_{j}")
                    o_ps[j] = (o0_t, o1_t)
                o0, o1 = o_ps[j]
                for i in range(2):
                    kt = 2 * kt2 + i
                    vo = kt * VSTRIDE
                    cs = slice(i * 512, i * 512 + w)
                    nc.tensor.matmul(
                        o0[0:HD + 1, 0:w],
                        lhsT=v_sb[:, vo:vo + HD + 1], rhs=e0[:, cs],
                        start=(kt == 0), stop=(kt == NKT - 1))
                    nc.tensor.matmul(
                        o1[0:HD + 1, 0:w],
                        lhsT=v_sb[:, vo + HD + 1:vo + 2 * (HD + 1)], rhs=e1[:, cs],
                        start=(kt == 0), stop=(kt == NKT - 1))
                if kt2 == NKT2 - 1:
                    # normalize this q-slice: rows 0:64 /= row 64.
                    # DVE ops are lane-local, so the reciprocal stays on
                    # partition 64, the broadcast bounces through DRAM, and
                    # h1's rows are DMA-shifted to partitions 64:128.
                    o0, o1 = o_ps.pop(j)
                    for h, op in ((0, o0), (1, o1)):
                        r = small.tile([P, 512], F32, tag=f"r{h}",
                                       name=f"r{h}_{b}_{j}")
                        nc.vector.reciprocal(r[HD:HD + 1, 0:w],
                                             op[HD:HD + 1, 0:w])
                        rb = dram_pool.tile([1, 512], F32, tag=f"rb{h}",
                                            name=f"rb{h}_{b}_{j}")
                        nc.sync.dma_start(out=rb[0:1, 0:w], in_=r[HD:HD + 1, 0:w])
                        bc = small.tile([HD, 512], F32, tag=f"bc{h}",
                                        name=f"bc{h}_{b}_{j}")
                        nc.sync.dma_start(out=bc[:, 0:w],
                                          in_=rb[0:1, 0:w].to_broadcast((HD, w)))
                        if h == 0:
                            nc.vector.tensor_mul(
                                ont_sb[0:HD, q0:q0 + w],
                                op[0:HD, 0:w], bc[0:HD, 0:w])
                        else:
                            tmp = small.tile([HD, 512], BF16, tag="tmp1",
                                             name=f"tmp1_{b}_{j}")
                            nc.vector.tensor_mul(
                                tmp[:, 0:w], op[0:HD, 0:w], bc[0:HD, 0:w])
                            nc.sync.dma_start(
                                out=ont_sb[HD:P, q0:q0 + w], in_=tmp[:, 0:w])
                    state["st_ready"] = (q0 + w) // P

            def emit_oproj():
                while state["st_emitted"] < state["st_ready"]:
                    st = state["st_emitted"]
                    ysb = y_pool.tile([P, D], F32, tag="ysb",
                                      name=f"ysb_{b}_{st}")
                    for half in range(2):
                        yp = ps_proj.tile([P, 512], F32, tag="pp",
                                          name=f"yp_{b}_{st}_{half}")
                        nc.tensor.matmul(
                            yp[:],
                            lhsT=ont_sb[:, st * P:(st + 1) * P],
                            rhs=wo_sb[:, half * 512:(half + 1) * 512],
                            start=True, stop=True)
                        nc.vector.tensor_copy(
                            ysb[:, half * 512:(half + 1) * 512], yp[:])
                    nc.sync.dma_start(out=y[b, st * P:(st + 1) * P, :], in_=ysb[:])
                    state["st_emitted"] += 1

            for i in range(len(units) + 1):
                if i < len(units):
                    emit_scores(units[i])
                if i > 0:
                    emit_av(units[i - 1])
                    emit_oproj()

    return nc


def _host_prep(x, cause_mask, effect_mask, intervention_strength,
               Wq, bq, Wk, bk, Wv, bv, Wo, bo):
    x = np.asarray(x, dtype=np.float32)
    cause_mask = np.asarray(cause_mask).astype(bool)
    effect_mask = np.asarray(effect_mask).astype(bool)
    s_int = float(np.asarray(intervention_strength))
    Wq = np.asarray(Wq, np.float32); bq = np.asarray(bq, np.float32)
    Wk = np.asarray(Wk, np.float32); bk = np.asarray(bk, np.float32)
    Wv = np.asarray(Wv, np.float32); bv = np.asarray(bv, np.float32)
    Wo = np.asarray(Wo, np.float32); bo = np.asarray(bo, np.float32)

    # host prep: sort tokens by cause_mask (descending) per batch
    perms, n_c1 = [], []
    for b in range(B):
        p = np.argsort(~cause_mask[b], kind="stable")
        perms.append(p)
        n_c1.append(int(cause_mask[b].sum()))
    xp = np.stack([x[b][perms[b]] for b in range(B)])          # [B, S, D]
    xt = np.ascontiguousarray(xp.transpose(0, 2, 1))           # [B, D, S]
    xtb = xt.astype(ml_dtypes.bfloat16)
    emfac = np.stack([
        1.0 - 0.5 * s_int * effect_mask[b][perms[b]].astype(np.float32)
        for b in range(B)])                                    # [B, S]
    emf = np.ascontiguousarray(
        np.broadcast_to(emfac[:, None, :], (B, P, S))).astype(np.float32)

    scale = 1.0 / np.sqrt(np.float32(HD))

    in_maps = []
    for c in range(NCORES):
        cols = slice(c * DPC, (c + 1) * DPC)
        in_maps.append({
            "xtb": xtb, "emf": emf,
            "wq": np.ascontiguousarray(Wq[:, cols] * scale).astype(ml_dtypes.bfloat16),
            "wk": np.ascontiguousarray(Wk[:, cols]).astype(ml_dtypes.bfloat16),
            "wvb": np.ascontiguousarray(Wv[:, cols]).astype(ml_dtypes.bfloat16),
            "wo": np.ascontiguousarray(Wo[cols, :]).astype(ml_dtypes.bfloat16),
            "bq": np.ascontiguousarray((bq[cols] * scale).reshape(DPC, 1)),
            "bk": np.ascontiguousarray(bk[cols].reshape(DPC, 1)),
        })

    bo_eff = bo + bv @ Wo
    return in_maps, perms, tuple(n_c1), bo_eff


def kernel(x, cause_mask, effect_mask, intervention_strength,
           Wq, bq, Wk, bk, Wv, bv, Wo, bo):
    global LAST_RESULTS
    in_maps, perms, n_c1, bo_eff = _host_prep(
        x, cause_mask, effect_mask, intervention_strength,
        Wq, bq, Wk, bk, Wv, bv, Wo, bo)
    nc = _build_program(n_c1)
    nc.finalize()
    LAST_RESULTS = run_bass_kernel_spmd(
        nc, in_maps, core_ids=list(range(NCORES)), trace=TRACE)

    y = np.zeros((B, S, D), np.float32)
    for c in range(NCORES):
        y += LAST_RESULTS.results[c]["y"]
    y += bo_eff[None, None, :]
    out = np.empty_like(y)
    for b in range(B):
        out[b][perms[b]] = y[b]  # undo the token sort
    return out


# revision 15
# speedup vs baseline: 160.5239x; 160.5239x over previous
"""Causal-intervention attention on 8 trn2 cores.

Sharding: head-parallel. Core c computes heads {2c, 2c+1} for BOTH batches
(so the SPMD program is identical on every core even though the
mask-dependent program structure differs per batch). Each core emits a
partial output y_c = ctx_c @ Wo[rows_c]; the host sums the 8 partials and
adds the (folded) bias.

Mask handling: tokens are sorted by cause_mask on the host (per batch).
scores * (1 - 0.5*s*cm[q]*em[k]) is then exact by using an em-scaled copy
of K^T (K2) for the cm=1 query block and plain K^T for the cm=0 block.

Softmax: exp without max subtraction (scores ~ N(0,1), max |s| < ~7, safe
in fp32). Denominator comes from a ones column appended to V (M=65 AV
matmul); normalization is applied to the attention output before the
output projection.

All matmul operands are bf16 (weights, x^T, Q^T/K^T, exp(S), V, O^T);
accumulation is fp32 in PSUM. fp32r streams at the same rate but its
self-loading LDWEIGHTS only accepts one semaphore wait, which Tile's
schedules exceed.
"""

import numpy as np
import ml_dtypes
from contextlib import ExitStack

import concourse.bass as bass
from concourse import bacc
import concourse.mybir as mybir
import concourse.tile as tile
from concourse.bass_utils import run_bass_kernel_spmd

B, S, D, H = 2, 2048, 1024, 16
HD = D // H  # 64
NCORES = 8
HPC = H // NCORES  # heads per core = 2
DPC = HPC * HD  # head-dim cols per core = 128
P = 128

F32 = mybir.dt.float32
BF16 = mybir.dt.bfloat16

NKT = S // P          # 16 key tiles of 128
NKT2 = NKT // 2       # 8 key-tile pairs
VSTRIDE = 2 * (HD + 1)  # V' storage stride per k-tile: [V_h0|1|V_h1|1] = 130

LAST_RESULTS = None  # BassKernelResults of the most recent run (for test.py)
TRACE = False


def _qslices(n_c1):
    """Query slices: breakpoints at multiples of 512 plus the cm=1/cm=0
    boundary. Each slice is (start, width, use_k2)."""
    pts = sorted(set([0, 512, 1024, 1536, 2048, int(n_c1)]))
    out = []
    for a, b in zip(pts[:-1], pts[1:]):
        out.append((a, b - a, b <= n_c1))
    return out


def _build_program(n_c1, repeat=1):
    """n_c1: tuple of per-batch cm=1 counts (compile-time structure).
    repeat>1 re-runs the whole computation (timing experiments only)."""
    nc = bacc.Bacc(None, target_bir_lowering=False)

    xtb = nc.declare_dram_parameter("xtb", [B, D, S], BF16, isOutput=False)
    wq = nc.declare_dram_parameter("wq", [D, DPC], BF16, isOutput=False)
    wk = nc.declare_dram_parameter("wk", [D, DPC], BF16, isOutput=False)
    wvb = nc.declare_dram_parameter("wvb", [D, DPC], BF16, isOutput=False)
    wo = nc.declare_dram_parameter("wo", [DPC, D], BF16, isOutput=False)
    bq = nc.declare_dram_parameter("bq", [DPC, 1], F32, isOutput=False)
    bk = nc.declare_dram_parameter("bk", [DPC, 1], F32, isOutput=False)
    emf = nc.declare_dram_parameter("emf", [B, P, S], F32, isOutput=False)
    y = nc.declare_dram_parameter("y", [B, S, D], F32, isOutput=True)

    with tile.TileContext(nc) as tc, ExitStack() as ctx:
        # ---- pools ----
        xtb_pool = ctx.enter_context(tc.tile_pool(name="xtb", bufs=10))
        wpool = ctx.enter_context(tc.tile_pool(name="w", bufs=1))
        emf_pool = ctx.enter_context(tc.tile_pool(name="emf", bufs=2))
        qk_pool = ctx.enter_context(tc.tile_pool(name="qk", bufs=2))
        v_pool = ctx.enter_context(tc.tile_pool(name="v", bufs=2))
        et_pool = ctx.enter_context(tc.tile_pool(name="et", bufs=4))
        small = ctx.enter_context(tc.tile_pool(name="small", bufs=3))
        y_pool = ctx.enter_context(tc.tile_pool(name="y", bufs=3))
        dram_pool = ctx.enter_context(tc.tile_pool(name="drp", bufs=2, space="DRAM"))
        ps_proj = ctx.enter_context(tc.tile_pool(name="psp", bufs=2, space="PSUM"))
        ps_s = ctx.enter_context(tc.tile_pool(name="pss", bufs=1, space="PSUM"))
        ps_o = ctx.enter_context(tc.tile_pool(name="pso", bufs=1, space="PSUM"))

        # ---- weights / biases (once) ----
        wqs, wks, wvbs = [], [], []
        for kt in range(D // P):
            tq = wpool.tile([P, DPC], BF16, tag=f"wq{kt}", name=f"wq_sb{kt}")
            nc.sync.dma_start(out=tq[:], in_=wq[kt * P:(kt + 1) * P, :])
            wqs.append(tq)
            tk = wpool.tile([P, DPC], BF16, tag=f"wk{kt}", name=f"wk_sb{kt}")
            nc.sync.dma_start(out=tk[:], in_=wk[kt * P:(kt + 1) * P, :])
            wks.append(tk)
            tv = wpool.tile([P, DPC], BF16, tag=f"wv{kt}", name=f"wv_sb{kt}")
            nc.sync.dma_start(out=tv[:], in_=wvb[kt * P:(kt + 1) * P, :])
            wvbs.append(tv)
        wo_sb = wpool.tile([P, D], BF16, tag="wo_sb")
        bq_sb = wpool.tile([P, 1], F32, tag="bq_sb")
        bk_sb = wpool.tile([P, 1], F32, tag="bk_sb")
        nc.sync.dma_start(out=wo_sb[:], in_=wo[:])
        nc.sync.dma_start(out=bq_sb[:], in_=bq[:])
        nc.sync.dma_start(out=bk_sb[:], in_=bk[:])

        for rep in range(repeat):
          for b in range(B):
            # ---- load x^T tiles (bf16) ----
            xtbs = []
            for kt in range(D // P):
                tb = xtb_pool.tile([P, S], BF16, tag="xtb")
                nc.sync.dma_start(out=tb[:], in_=xtb[b, kt * P:(kt + 1) * P, :])
                xtbs.append(tb)
            emf_sb = emf_pool.tile([P, S], F32, tag="emf_sb")
            nc.sync.dma_start(out=emf_sb[:], in_=emf[b])

            # ---- Q^T / K^T projections (d-major [128, S], bf16) ----
            qt_sb = qk_pool.tile([P, S], BF16, tag="qt_sb")
            kt_sb = qk_pool.tile([P, S], BF16, tag="kt_sb")
            k2t_sb = qk_pool.tile([P, S], BF16, tag="k2t_sb")
            ont_sb = qk_pool.tile([P, S], BF16, tag="ont_sb")
            for ns in range(4):
                cs = slice(ns * 512, (ns + 1) * 512)
                psq = ps_proj.tile([P, 512], F32, tag="pp", name=f"psq_{b}_{ns}")
                for kt in range(D // P):
                    nc.tensor.matmul(
                        psq[:], lhsT=wqs[kt][:], rhs=xtbs[kt][:, cs],
                        start=(kt == 0), stop=(kt == D // P - 1))
                nc.vector.tensor_scalar_add(qt_sb[:, cs], psq[:], bq_sb[:])
                psk = ps_proj.tile([P, 512], F32, tag="pp", name=f"psk_{b}_{ns}")
                for kt in range(D // P):
                    nc.tensor.matmul(
                        psk[:], lhsT=wks[kt][:], rhs=xtbs[kt][:, cs],
                        start=(kt == 0), stop=(kt == D // P - 1))
                nc.vector.tensor_scalar_add(kt_sb[:, cs], psk[:], bk_sb[:])
                # K2 = (K + bk) * emfac  (em-scaled copy)
                nc.vector.scalar_tensor_tensor(
                    out=k2t_sb[:, cs], in0=psk[:], scalar=bk_sb[:],
                    in1=emf_sb[:, cs],
                    op0=mybir.AluOpType.add, op1=mybir.AluOpType.mult)

            # ---- V projection (s-major, with ones cols): V'[128, 16*130] ----
            v_sb = v_pool.tile([P, NKT * VSTRIDE], BF16, tag="v_sb")
            nc.vector.memset(v_sb[:], 1.0)
            for st in range(NKT):
                psv = ps_proj.tile([P, 512], F32, tag="pp", name=f"psv_{b}_{st}")
                for kt in range(D // P):
                    nc.tensor.matmul(
                        psv[:, 0:DPC],
                        lhsT=xtbs[kt][:, st * P:(st + 1) * P],
                        rhs=wvbs[kt][:],
                        start=(kt == 0), stop=(kt == D // P - 1))
                o = st * VSTRIDE
                nc.vector.tensor_copy(v_sb[:, o:o + HD], psv[:, 0:HD])
                nc.vector.tensor_copy(v_sb[:, o + HD + 1:o + 2 * HD + 1],
                                      psv[:, HD:2 * HD])

            # ---- attention (+ interleaved output projection) ----
            qsl = _qslices(n_c1[b])
            units = [(j, kt2) for j in range(len(qsl)) for kt2 in range(NKT2)]
            o_ps = {}   # per live qslice: (o0, o1) psum tiles
            pend = {}   # unit -> (e0, e1)
            state = {"st_ready": 0, "st_emitted": 0}

            def emit_scores(u):
                j, kt2 = u
                q0, w, use_k2 = qsl[j]
                src = k2t_sb if use_k2 else kt_sb
                sp0 = ps_s.tile([P, 1024], F32, tag="sp0", name=f"sp0_{b}_{j}_{kt2}")
                sp1 = ps_s.tile([P, 1024], F32, tag="sp1", name=f"sp1_{b}_{j}_{kt2}")
                for i in range(2):
                    kt = 2 * kt2 + i
                    ks = slice(kt * P, (kt + 1) * P)
                    nc.tensor.matmul(
                        sp0[:, i * 512:i * 512 + w],
                        lhsT=src[0:HD, ks], rhs=qt_sb[0:HD, q0:q0 + w],
                        start=True, stop=True, tile_position=(0, 0))
                    nc.tensor.matmul(
                        sp1[:, i * 512:i * 512 + w],
                        lhsT=src[HD:P, ks], rhs=qt_sb[HD:P, q0:q0 + w],
                        start=True, stop=True, tile_position=(64, 0))
                e0 = et_pool.tile([P, 1024], BF16, tag="e0", name=f"e0_{b}_{j}_{kt2}")
                e1 = et_pool.tile([P, 1024], BF16, tag="e1", name=f"e1_{b}_{j}_{kt2}")
                for i in range(2):
                    cs = slice(i * 512, i * 512 + w)
                    nc.scalar.activation(e0[:, cs], sp0[:, cs],
                                         mybir.ActivationFunctionType.Exp)
                    nc.scalar.activation(e1[:, cs], sp1[:, cs],
                                         mybir.ActivationFunctionType.Exp)
                pend[u] = (e0, e1)

            def emit_av(u):
                j, kt2 = u
                q0, w, _ = qsl[j]
                e0, e1 = pend.pop(u)
                if j not in o_ps:
                    o0_t = ps_o.tile([HD + 1, 512], F32, tag="o0", name=f"o0_{b}_{j}")
                    o1_t = ps_o.tile([HD + 1, 512], F32, tag="o1", name=f"o1_{b}_{j}")
                    o_ps[j] = (o0_t, o1_t)
                o0, o1 = o_ps[j]
                for i in range(2):
                    kt = 2 * kt2 + i
                    vo = kt * VSTRIDE
                    cs = slice(i * 512, i * 512 + w)
                    nc.tensor.matmul(
                        o0[0:HD + 1, 0:w],
                        lhsT=v_sb[:, vo:vo + HD + 1], rhs=e0[:, cs],
                        start=(kt == 0), stop=(kt == NKT - 1))
                    nc.tensor.matmul(
                        o1[0:HD + 1, 0:w],
                        lhsT=v_sb[:, vo + HD + 1:vo + 2 * (HD + 1)], rhs=e1[:, cs],
                        start=(kt == 0), stop=(kt == NKT - 1))
                if kt2 == NKT2 - 1:
                    # normalize this q-slice: rows 0:64 /= row 64.
                    # DVE ops are lane-local, so the reciprocal stays on
                    # partition 64, the broadcast bounces through DRAM, and
                    # h1's rows are DMA-shifted to partitions 64:128.
                    o0, o1 = o_ps.pop(j)
                    for h, op in ((0, o0), (1, o1)):
                        r = small.tile([P, 512], F32, tag=f"r{h}",
                                       name=f"r{h}_{b}_{j}")
                        nc.vector.reciprocal(r[HD:HD + 1, 0:w],
                                             op[HD:HD + 1, 0:w])
                        rb = dram_pool.tile([1, 512], F32, tag=f"rb{h}",
                                            name=f"rb{h}_{b}_{j}")
                        nc.sync.dma_start(out=rb[0:1, 0:w], in_=r[HD:HD + 1, 0:w])
                        bc = small.tile([HD, 512], F32, tag=f"bc{h}",
                                        name=f"bc{h}_{b}_{j}")
                        nc.sync.dma_start(out=bc[:, 0:w],
                                          in_=rb[0:1, 0:w].to_broadcast((HD, w)))
                        if h == 0:
                            nc.vector.tensor_mul(
                                ont_sb[0:HD, q0:q0 + w],
                                op[0:HD, 0:w], bc[0:HD, 0:w])
                        else:
                            tmp = small.tile([HD, 512], BF16, tag="tmp1",
                                             name=f"tmp1_{b}_{j}")
                            nc.vector.tensor_mul(
                                tmp[:, 0:w], op[0:HD, 0:w], bc[0:HD, 0:w])
                            nc.sync.dma_start(
                                out=ont_sb[HD:P, q0:q0 + w], in_=tmp[:, 0:w])
                    state["st_ready"] = (q0 + w) // P

            def emit_oproj():
                while state["st_emitted"] < state["st_ready"]:
                    st = state["st_emitted"]
                    ysb = y_pool.tile([P, D], F32, tag="ysb",
                                      name=f"ysb_{b}_{st}")
                    for half in range(2):
                        yp = ps_proj.tile([P, 512], F32, tag="pp",
                                          name=f"yp_{b}_{st}_{half}")
                        nc.tensor.matmul(
                            yp[:],
                            lhsT=ont_sb[:, st * P:(st + 1) * P],
                            rhs=wo_sb[:, half * 512:(half + 1) * 512],
                            start=True, stop=True)
                        nc.vector.tensor_copy(
                            ysb[:, half * 512:(half + 1) * 512], yp[:])
                    nc.sync.dma_start(out=y[b, st * P:(st + 1) * P, :], in_=ysb[:])
                    state["st_emitted"] += 1

            for i in range(len(units) + 1):
                if i < len(units):
                    emit_scores(units[i])
                if i > 0:
                    emit_av(units[i - 1])
                    emit_oproj()

    return nc


def _host_prep(x, cause_mask, effect_mask, intervention_strength,
               Wq, bq, Wk, bk, Wv, bv, Wo, bo):
    x = np.asarray(x, dtype=np.float32)
    cause_mask = np.asarray(cause_mask).astype(bool)
    effect_mask = np.asarray(effect_mask).astype(bool)
    s_int = float(np.asarray(intervention_strength))
    Wq = np.asarray(Wq, np.float32); bq = np.asarray(bq, np.float32)
    Wk = np.asarray(Wk, np.float32); bk = np.asarray(bk, np.float32)
    Wv = np.asarray(Wv, np.float32); bv = np.asarray(bv, np.float32)
    Wo = np.asarray(Wo, np.float32); bo = np.asarray(bo, np.float32)

    # host prep: sort tokens by cause_mask (descending) per batch
    perms, n_c1 = [], []
    for b in range(B):
        p = np.argsort(~cause_mask[b], kind="stable")
        perms.append(p)
        n_c1.append(int(cause_mask[b].sum()))
    xp = np.stack([x[b][perms[b]] for b in range(B)])          # [B, S, D]
    xt = np.ascontiguousarray(xp.transpose(0, 2, 1))           # [B, D, S]
    xtb = xt.astype(ml_dtypes.bfloat16)
    emfac = np.stack([
        1.0 - 0.5 * s_int * effect_mask[b][perms[b]].astype(np.float32)
        for b in range(B)])                                    # [B, S]
    emf = np.ascontiguousarray(
        np.broadcast_to(emfac[:, None, :], (B, P, S))).astype(np.float32)

    scale = 1.0 / np.sqrt(np.float32(HD))

    in_maps = []
    for c in range(NCORES):
        cols = slice(c * DPC, (c + 1) * DPC)
        in_maps.append({
            "xtb": xtb, "emf": emf,
            "wq": np.ascontiguousarray(Wq[:, cols] * scale).astype(ml_dtypes.bfloat16),
            "wk": np.ascontiguousarray(Wk[:, cols]).astype(ml_dtypes.bfloat16),
            "wvb": np.ascontiguousarray(Wv[:, cols]).astype(ml_dtypes.bfloat16),
            "wo": np.ascontiguousarray(Wo[cols, :]).astype(ml_dtypes.bfloat16),
            "bq": np.ascontiguousarray((bq[cols] * scale).reshape(DPC, 1)),
            "bk": np.ascontiguousarray(bk[cols].reshape(DPC, 1)),
        })

    bo_eff = bo + bv @ Wo
    return in_maps, perms, tuple(n_c1), bo_eff


def kernel(x, cause_mask, effect_mask, intervention_strength,
           Wq, bq, Wk, bk, Wv, bv, Wo, bo):
    global LAST_RESULTS
    in_maps, perms, n_c1, bo_eff = _host_prep(
        x, cause_mask, effect_mask, intervention_strength,
        Wq, bq, Wk, bk, Wv, bv, Wo, bo)
    nc = _build_program(n_c1)
    nc.finalize()
    LAST_RESULTS = run_bass_kernel_spmd(
        nc, in_maps, core_ids=list(range(NCORES)), trace=TRACE)

    y = np.zeros((B, S, D), np.float32)
    for c in range(NCORES):
        y += LAST_RESULTS.results[c]["y"]
    y += bo_eff[None, None, :]
    out = np.empty_like(y)
    for b in range(B):
        out[b][perms[b]] = y[b]  # undo the token sort
    return out
